# revision 11
# baseline (speedup 1.0000x reference)
"""Trainium2 Bass kernel for BiochemicalDynamics.

Reference computation (f32):
    Ax    = A @ x                                   # [N, DIM]
    s     = R * rowsum(x * Ax)                      # [N, 1]
    out   = F - B*x - s                             # [N, DIM]

Strategy: compute Y^T = (A_local @ x)^T directly on the TensorEngine by
streaming A (host-side pre-transposed, fp8) as the MOVING operand
against stationary x row-chunks:

    Y^T[d, m] = sum_kc matmul(lhsT = x[kc*128:(kc+1)*128, :],     # [K, M=64]
                              rhs  = A^T[kc*128:(kc+1)*128, m])   # [K, N]

accumulated over all 64 K-chunks into a PSUM region [64, 1024].
fp8 DoubleRow packs two K-chunks per instruction (K=256), giving the PE
enough column rate to hide entirely under the A DMA stream.

The per-row dot s_i = R * <x_i, Y_i> needs a PARTITION-axis reduction
of z = x^T .* Y^T, done with tiny ones-vector matmuls (lhsT =
z[:, stripe], rhs = ones[64,1]) that land s in natural [128, .] layout
for the ScalarE epilogue out = F - B*x - s.

A is streamed m-major in 4 quarters (256 output rows each): quarter q's
Y^T finishes while quarter q+1 is still streaming, so its reduction,
epilogue and output store all hide under the DMA stream — only the last
quarter's (short) chain sits in the tail.

A is quantized to fp8_e4m3 host-side: its rounding error is random-sign
and averages out over the 8192-term row reductions (measured ~1.6e-3
max rel err vs the 2e-2 gate) while halving HBM traffic vs fp16 —
this kernel is DMA-bound on A (8MB/core at ~341 GB/s ~= 23.5us).

Sharding: row-shard A (1024 rows/core); every core gets the full x.
No cross-core communication.
"""

import sys

import numpy as np

for _p in ("/opt/trn_rl_repo", "/root/.axon_site/_ro/trn_rl_repo"):
    if _p not in sys.path:
        sys.path.append(_p)

N = 8192
DIM = 64
NCORES = 8
ROWS = N // NCORES       # 1024 rows of A per core
P = 128
NSTRIPES = ROWS // P     # 8 row-stripes per core
KC = N // P              # 64 contraction chunks of 128
KP = KC // 2             # 32 DoubleRow chunk-pairs
NQ = 4                   # m-quarters (256 output rows each)
QW = ROWS // NQ          # 256

F_CONST = 1.0
B_CONST = 0.1
R_CONST = 0.01

# Per-quarter A DMA schedule in kc units. Quarter 0 ramps up so the
# first matmuls start early; later quarters use 1MB transfers.
Q_CHUNKS = (
    (8, 24, 32),            # quarter 0 (small first chunk -> early PE start)
    (32, 32),               # quarter 1 (issued on the ACT ring)
    (32, 32),
    (32, 16, 8, 8),         # taper: the PE catches the stream fast at the end
)
assert all(sum(ch) == KC for ch in Q_CHUNKS)

_CACHE = {}


def _build_nc():
    import concourse.mybir as mybir
    import concourse.tile as tile
    from concourse import bacc

    f32 = mybir.dt.float32
    bf16 = mybir.dt.bfloat16
    f8 = mybir.dt.float8e4

    nc = bacc.Bacc(
        trn_type="TRN2", target_bir_lowering=False, debug=False, num_devices=NCORES
    )

    # at[q, p, kc, j] = A[rows_c[q*256 + j], kc*128 + p]  (pre-transposed fp8 A)
    at = nc.dram_tensor("at", [NQ, P, KC, QW], f8, kind="ExternalInput")
    # xs[p, kc, d] = x[kc*128 + p, d]          (stationary chunks, fp8)
    xs = nc.dram_tensor("xs", [P, KC, DIM], f8, kind="ExternalInput")
    # xtd[d, m] = x[rows_c[m], d]              (bf16, for the rowwise dot)
    xtd = nc.dram_tensor("xtd", [DIM, ROWS], bf16, kind="ExternalInput")
    # xl[p, s*64+d] = x[rows_c[s*128+p], d]    (bf16, for the epilogue)
    xl = nc.dram_tensor("xl", [P, NSTRIPES * DIM], bf16, kind="ExternalInput")
    out = nc.dram_tensor("out", [P, NSTRIPES * DIM], f32, kind="ExternalOutput")

    mult = mybir.AluOpType.mult
    DR = mybir.MatmulPerfMode.DoubleRow

    with tile.TileContext(nc) as tc:
        with (
            tc.tile_pool(name="xpool", bufs=1) as xpool,
            tc.tile_pool(name="spool", bufs=1) as spool,
            tc.tile_pool(name="psum", bufs=1, space="PSUM") as psum_pool,
        ):
            # Small loads on the ACT HWDGE ring so they don't stall the
            # A stream on the SP ring. First xs piece is small so the
            # first matmul can start ASAP.
            xs_sb = xpool.tile([P, KC, DIM], f8)
            nc.scalar.dma_start(out=xs_sb[:, 0:4, :], in_=xs[:, 0:4, :])
            nc.scalar.dma_start(out=xs_sb[:, 4:, :], in_=xs[:, 4:, :])
            xtd_sb = xpool.tile([DIM, ROWS], bf16)
            nc.scalar.dma_start(out=xtd_sb[:], in_=xtd[:])
            xl_sb = xpool.tile([P, NSTRIPES * DIM], bf16)
            nc.scalar.dma_start(out=xl_sb[:], in_=xl[:])
            ones_sb = xpool.tile([DIM, 1], bf16)
            nc.any.memset(ones_sb[:], 1.0)

            # A stream, quarter-major, split across BOTH HWDGE rings
            # (quarter 1 on the ACT ring) — each ring stalls issuing past
            # ~13 queued DMAs, so keep both well under that.
            at_sb = [
                xpool.tile([P, KC, QW], f8, name=f"at_sb{q}", tag=f"at{q}")
                for q in range(NQ)
            ]
            for q in range(NQ):
                ring = nc.scalar if q == 1 else nc.sync
                o = 0
                for w in Q_CHUNKS[q]:
                    ring.dma_start(
                        out=at_sb[q][:, o : o + w, :], in_=at[q, :, o : o + w, :]
                    )
                    o += w

            # Y^T accumulation: [64, 1024] f32 PSUM (2 banks).
            yt_ps = psum_pool.tile([DIM, ROWS], f32, tag="yt")
            z_sb = spool.tile([DIM, ROWS], bf16, tag="z")
            s_ps = psum_pool.tile([P, NSTRIPES], f32, tag="s")
            v_sb = spool.tile([P, NSTRIPES], f32, tag="v")
            o_sb = spool.tile([P, NSTRIPES * DIM], f32, tag="o")

            def emit_z(mlo, mhi):
                # z[d, m] = (xtd * R) * Y^T  -> bf16
                nc.vector.scalar_tensor_tensor(
                    z_sb[:, mlo:mhi],
                    xtd_sb[:, mlo:mhi],
                    R_CONST,
                    yt_ps[:, mlo:mhi],
                    op0=mult,
                    op1=mult,
                )

            def pe_reduce(stripes):
                # s[p, s] = sum_d z[d, s*128 + p]
                for s in stripes:
                    nc.tensor.matmul(
                        s_ps[:, s : s + 1],
                        z_sb[:, s * P : (s + 1) * P],
                        ones_sb[:],
                        start=True,
                        stop=True,
                    )

            def scalar_epilogue(stripes):
                # v = F - s, then out = Identity(xl * -B + v)
                s0, s1 = stripes[0], stripes[-1] + 1
                nc.scalar.activation(
                    v_sb[:, s0:s1], s_ps[:, s0:s1],
                    mybir.ActivationFunctionType.Copy,
                    bias=F_CONST, scale=-1.0,
                )
                for s in stripes:
                    nc.scalar.activation(
                        o_sb[:, s * DIM : (s + 1) * DIM],
                        xl_sb[:, s * DIM : (s + 1) * DIM],
                        mybir.ActivationFunctionType.Identity,
                        bias=v_sb[:, s : s + 1],
                        scale=-B_CONST,
                    )

            def store(q, engine):
                engine.dma_start(
                    out=out[:, q * 2 * DIM : (q + 1) * 2 * DIM],
                    in_=o_sb[:, q * 2 * DIM : (q + 1) * 2 * DIM],
                )

            for q in range(NQ):
                for c in range(KP):
                    nc.tensor.matmul(
                        yt_ps[:, q * QW : (q + 1) * QW],
                        xs_sb[:, 2 * c : 2 * c + 2, :],
                        at_sb[q][:, 2 * c : 2 * c + 2, :],
                        start=(c == 0),
                        stop=(c == KP - 1),
                        perf_mode=DR,
                    )
                if q < NQ - 1:
                    emit_z(q * QW, (q + 1) * QW)
                    # Emit quarter q-1's PE reduction AFTER quarter q's
                    # matmuls so the PE never stalls on the DVE mid-stream.
                    if q > 0:
                        pe_reduce((2 * (q - 1), 2 * (q - 1) + 1))
                        scalar_epilogue((2 * (q - 1), 2 * (q - 1) + 1))
                        store(q - 1, nc.scalar)
                else:
                    # Last quarter: drain quarter q-1, then pipeline the
                    # final chain per stripe across DVE/PE/ScalarE, with
                    # the store on the (now idle) SP ring.
                    pe_reduce((2 * (q - 1), 2 * (q - 1) + 1))
                    scalar_epilogue((2 * (q - 1), 2 * (q - 1) + 1))
                    store(q - 1, nc.scalar)
                    for s in (2 * q, 2 * q + 1):
                        emit_z(s * P, (s + 1) * P)
                        pe_reduce((s,))
                        scalar_epilogue((s,))
                    store(q, nc.sync)

    nc.finalize()
    return nc


def _get_nc():
    if "nc" not in _CACHE:
        _CACHE["nc"] = _build_nc()
    return _CACHE["nc"]


def _make_in_maps(x, A):
    import ml_dtypes

    f8 = ml_dtypes.float8_e4m3
    x = np.ascontiguousarray(np.asarray(x, dtype=np.float32))
    A = np.asarray(A, dtype=np.float32)

    # xs[p, kc, d] = x[kc*128 + p, d]
    xs = np.ascontiguousarray(
        x.reshape(KC, P, DIM).transpose(1, 0, 2)
    ).astype(f8)

    in_maps = []
    for c in range(NCORES):
        rows = slice(c * ROWS, (c + 1) * ROWS)
        xc = x[rows]
        # at[q, p, kc, j] = A[rows[q*256 + j], kc*128 + p]
        atq = A[rows].T.astype(f8)                       # [8192, 1024] fp8
        at = np.ascontiguousarray(
            atq.reshape(KC, P, NQ, QW).transpose(2, 1, 0, 3)
        )
        in_maps.append(
            {
                "at": at,
                "xs": xs,
                "xtd": np.ascontiguousarray(xc.T).astype(ml_dtypes.bfloat16),
                "xl": np.ascontiguousarray(
                    xc.reshape(NSTRIPES, P, DIM).transpose(1, 0, 2)
                ).reshape(P, NSTRIPES * DIM).astype(ml_dtypes.bfloat16),
            }
        )
    return in_maps


def run_sharded(x, A, trace=False, **kwargs):
    """Run the SPMD bass kernel; returns (full_output, BassKernelResults)."""
    from concourse.bass_utils import run_bass_kernel_spmd

    nc = _get_nc()
    res = run_bass_kernel_spmd(
        nc, _make_in_maps(x, A), core_ids=list(range(NCORES)), trace=trace, **kwargs
    )
    full = np.concatenate(
        [
            res.results[c]["out"]
            .reshape(P, NSTRIPES, DIM)
            .transpose(1, 0, 2)
            .reshape(ROWS, DIM)
            for c in range(NCORES)
        ],
        axis=0,
    )
    return full.astype(np.float32, copy=False), res


def kernel(t, x, A):
    out, _ = run_sharded(x, A)
    return out


# revision 14
# speedup vs baseline: 1.3669x; 1.3669x over previous
"""Trainium2 Bass kernel for BiochemicalDynamics.

Reference computation (f32):
    Ax    = A @ x                                   # [N, DIM]
    s     = R * rowsum(x * Ax)                      # [N, 1]
    out   = F - B*x - s                             # [N, DIM]

Strategy: compute Y^T = (A_local @ x)^T directly on the TensorEngine by
streaming A (host-side pre-transposed, fp8) as the MOVING operand
against stationary x row-chunks:

    Y^T[d, m] = sum_kc matmul(lhsT = x[kc*128:(kc+1)*128, :],     # [K, M=64]
                              rhs  = A^T[kc*128:(kc+1)*128, m])   # [K, N]

accumulated over all 64 K-chunks into a PSUM region [64, 1024].
fp8 DoubleRow packs two K-chunks per instruction (K=256), giving the PE
enough column rate to hide entirely under the A DMA stream.

The per-row dot s_i = R * <x_i, Y_i> needs a PARTITION-axis reduction
of z = x^T .* Y^T, done with tiny ones-vector matmuls (lhsT =
z[:, stripe], rhs = ones[64,1]) that land s in natural [128, .] layout
for the ScalarE epilogue out = F - B*x - s.

A is streamed m-major in 4 quarters (256 output rows each): quarter q's
Y^T finishes while quarter q+1 is still streaming, so its reduction,
epilogue and output store all hide under the DMA stream — only the last
quarter's (short) chain sits in the tail.

A is quantized to fp8_e4m3 host-side: its rounding error is random-sign
and averages out over the 8192-term row reductions (measured ~1.6e-3
max rel err vs the 2e-2 gate) while halving HBM traffic vs fp16 —
this kernel is DMA-bound on A (8MB/core at ~341 GB/s ~= 23.5us).

Sharding: row-shard A (1024 rows/core); every core gets the full x.
No cross-core communication.
"""

import sys

import numpy as np

for _p in ("/opt/trn_rl_repo", "/root/.axon_site/_ro/trn_rl_repo"):
    if _p not in sys.path:
        sys.path.append(_p)

N = 8192
DIM = 64
NCORES = 8
ROWS = N // NCORES       # 1024 rows of A per core
P = 128
NSTRIPES = ROWS // P     # 8 row-stripes per core
KC = N // P              # 64 contraction chunks of 128
KP = KC // 2             # 32 DoubleRow chunk-pairs
NQ = 4                   # m-quarters (256 output rows each)
QW = ROWS // NQ          # 256

F_CONST = 1.0
B_CONST = 0.1
R_CONST = 0.01

# Per-quarter A DMA schedule in kc units. Quarter 0 ramps up so the
# first matmuls start early; later quarters use 1MB transfers.
Q_CHUNKS = (
    (8, 24, 32),            # quarter 0 (small first chunk -> early PE start)
    (32, 32),
    (32, 32),
    (32, 16, 8, 8),         # taper: the PE catches the stream fast at the end
)
assert all(sum(ch) == KC for ch in Q_CHUNKS)

_CACHE = {}


def _build_nc():
    import concourse.mybir as mybir
    import concourse.tile as tile
    from concourse import bacc

    f32 = mybir.dt.float32
    bf16 = mybir.dt.bfloat16
    f8 = mybir.dt.float8e4

    nc = bacc.Bacc(
        trn_type="TRN2", target_bir_lowering=False, debug=False, num_devices=NCORES
    )

    # at[q, p, kc, j] = A[rows_c[q*256 + j], kc*128 + p]  (pre-transposed fp8 A)
    at = nc.dram_tensor("at", [NQ, P, KC, QW], f8, kind="ExternalInput")
    # xs[p, kc, d] = x[kc*128 + p, d]          (stationary chunks, fp8)
    xs = nc.dram_tensor("xs", [P, KC, DIM], f8, kind="ExternalInput")
    # xtd[d, m] = x[rows_c[m], d]              (bf16, for the rowwise dot)
    xtd = nc.dram_tensor("xtd", [DIM, ROWS], bf16, kind="ExternalInput")
    # xl[p, s*64+d] = x[rows_c[s*128+p], d]    (bf16, for the epilogue)
    xl = nc.dram_tensor("xl", [P, NSTRIPES * DIM], bf16, kind="ExternalInput")
    out = nc.dram_tensor("out", [P, NSTRIPES * DIM], f32, kind="ExternalOutput")

    mult = mybir.AluOpType.mult
    DR = mybir.MatmulPerfMode.DoubleRow

    with tile.TileContext(nc) as tc:
        with (
            tc.tile_pool(name="xpool", bufs=1) as xpool,
            tc.tile_pool(name="spool", bufs=1) as spool,
            tc.tile_pool(name="psum", bufs=1, space="PSUM") as psum_pool,
        ):
            # ALL loads go through the single SP HWDGE ring, in FIFO
            # order, so every DMAHW sem lane's previous user is an
            # early-completing load — a lane shared with a late-gated
            # DMA stalls the ring for several us (the lanes are assigned
            # round-robin across ALL HWDGE DMAs, both rings).
            xs_sb = xpool.tile([P, KC, DIM], f8)
            xtd_sb = xpool.tile([DIM, ROWS], bf16)
            xl_sb = xpool.tile([P, NSTRIPES * DIM], bf16)
            ones_sb = xpool.tile([DIM, 1], bf16)
            nc.any.memset(ones_sb[:], 1.0)
            at_sb = [
                xpool.tile([P, KC, QW], f8, name=f"at_sb{q}", tag=f"at{q}")
                for q in range(NQ)
            ]

            # First the stationary piece the first matmuls need, then the
            # first A chunk, then the rest of the small loads, then the
            # A stream.
            nc.sync.dma_start(out=xs_sb[:, 0:4, :], in_=xs[:, 0:4, :])
            nc.sync.dma_start(
                out=at_sb[0][:, 0 : Q_CHUNKS[0][0], :],
                in_=at[0, :, 0 : Q_CHUNKS[0][0], :],
            )
            nc.sync.dma_start(out=xs_sb[:, 4:, :], in_=xs[:, 4:, :])
            nc.sync.dma_start(out=xtd_sb[:], in_=xtd[:])
            nc.sync.dma_start(out=xl_sb[:], in_=xl[:])
            for q in range(NQ):
                o = Q_CHUNKS[0][0] if q == 0 else 0
                for w in Q_CHUNKS[q][1 if q == 0 else 0 :]:
                    nc.sync.dma_start(
                        out=at_sb[q][:, o : o + w, :], in_=at[q, :, o : o + w, :]
                    )
                    o += w

            # Y^T accumulation: [64, 1024] f32 PSUM (2 banks).
            yt_ps = psum_pool.tile([DIM, ROWS], f32, tag="yt")
            z_sb = spool.tile([DIM, ROWS], bf16, tag="z")
            s_ps = psum_pool.tile([P, NSTRIPES], f32, tag="s")
            v_sb = spool.tile([P, NSTRIPES], f32, tag="v")
            o_sb = spool.tile([P, NSTRIPES * DIM], f32, tag="o")

            def emit_z(mlo, mhi):
                # z[d, m] = (xtd * R) * Y^T  -> bf16
                nc.vector.scalar_tensor_tensor(
                    z_sb[:, mlo:mhi],
                    xtd_sb[:, mlo:mhi],
                    R_CONST,
                    yt_ps[:, mlo:mhi],
                    op0=mult,
                    op1=mult,
                )

            def pe_reduce(stripes):
                # s[p, s] = sum_d z[d, s*128 + p]
                for s in stripes:
                    nc.tensor.matmul(
                        s_ps[:, s : s + 1],
                        z_sb[:, s * P : (s + 1) * P],
                        ones_sb[:],
                        start=True,
                        stop=True,
                    )

            def scalar_epilogue(stripes):
                # v = F - s, then out = Identity(xl * -B + v)
                s0, s1 = stripes[0], stripes[-1] + 1
                nc.scalar.activation(
                    v_sb[:, s0:s1], s_ps[:, s0:s1],
                    mybir.ActivationFunctionType.Copy,
                    bias=F_CONST, scale=-1.0,
                )
                for s in stripes:
                    nc.scalar.activation(
                        o_sb[:, s * DIM : (s + 1) * DIM],
                        xl_sb[:, s * DIM : (s + 1) * DIM],
                        mybir.ActivationFunctionType.Identity,
                        bias=v_sb[:, s : s + 1],
                        scale=-B_CONST,
                    )

            def store(q, engine):
                engine.dma_start(
                    out=out[:, q * 2 * DIM : (q + 1) * 2 * DIM],
                    in_=o_sb[:, q * 2 * DIM : (q + 1) * 2 * DIM],
                )

            for q in range(NQ):
                for c in range(KP):
                    nc.tensor.matmul(
                        yt_ps[:, q * QW : (q + 1) * QW],
                        xs_sb[:, 2 * c : 2 * c + 2, :],
                        at_sb[q][:, 2 * c : 2 * c + 2, :],
                        start=(c == 0),
                        stop=(c == KP - 1),
                        perf_mode=DR,
                    )
                if q < NQ - 1:
                    emit_z(q * QW, (q + 1) * QW)
                    # Emit quarter q-1's PE reduction AFTER quarter q's
                    # matmuls so the PE never stalls on the DVE mid-stream.
                    # Mid-stream stores go via GPSIMD (SWDGE): their late
                    # gating must stay off the HWDGE sem lanes.
                    if q > 0:
                        pe_reduce((2 * (q - 1), 2 * (q - 1) + 1))
                        scalar_epilogue((2 * (q - 1), 2 * (q - 1) + 1))
                        store(q - 1, nc.gpsimd)
                else:
                    # Last quarter: drain quarter q-1, then pipeline the
                    # final chain per stripe across DVE/PE/ScalarE, with
                    # the store last on the SP ring.
                    pe_reduce((2 * (q - 1), 2 * (q - 1) + 1))
                    scalar_epilogue((2 * (q - 1), 2 * (q - 1) + 1))
                    store(q - 1, nc.gpsimd)
                    for s in (2 * q, 2 * q + 1):
                        emit_z(s * P, (s + 1) * P)
                        pe_reduce((s,))
                        scalar_epilogue((s,))
                    store(q, nc.sync)

    nc.finalize()
    return nc


def _get_nc():
    if "nc" not in _CACHE:
        _CACHE["nc"] = _build_nc()
    return _CACHE["nc"]


def _make_in_maps(x, A):
    import ml_dtypes

    f8 = ml_dtypes.float8_e4m3
    x = np.ascontiguousarray(np.asarray(x, dtype=np.float32))
    A = np.asarray(A, dtype=np.float32)

    # xs[p, kc, d] = x[kc*128 + p, d]
    xs = np.ascontiguousarray(
        x.reshape(KC, P, DIM).transpose(1, 0, 2)
    ).astype(f8)

    in_maps = []
    for c in range(NCORES):
        rows = slice(c * ROWS, (c + 1) * ROWS)
        xc = x[rows]
        # at[q, p, kc, j] = A[rows[q*256 + j], kc*128 + p]
        atq = A[rows].T.astype(f8)                       # [8192, 1024] fp8
        at = np.ascontiguousarray(
            atq.reshape(KC, P, NQ, QW).transpose(2, 1, 0, 3)
        )
        in_maps.append(
            {
                "at": at,
                "xs": xs,
                "xtd": np.ascontiguousarray(xc.T).astype(ml_dtypes.bfloat16),
                "xl": np.ascontiguousarray(
                    xc.reshape(NSTRIPES, P, DIM).transpose(1, 0, 2)
                ).reshape(P, NSTRIPES * DIM).astype(ml_dtypes.bfloat16),
            }
        )
    return in_maps


def run_sharded(x, A, trace=False, **kwargs):
    """Run the SPMD bass kernel; returns (full_output, BassKernelResults)."""
    from concourse.bass_utils import run_bass_kernel_spmd

    nc = _get_nc()
    res = run_bass_kernel_spmd(
        nc, _make_in_maps(x, A), core_ids=list(range(NCORES)), trace=trace, **kwargs
    )
    full = np.concatenate(
        [
            res.results[c]["out"]
            .reshape(P, NSTRIPES, DIM)
            .transpose(1, 0, 2)
            .reshape(ROWS, DIM)
            for c in range(NCORES)
        ],
        axis=0,
    )
    return full.astype(np.float32, copy=False), res


def kernel(t, x, A):
    out, _ = run_sharded(x, A)
    return out
